# revision 46
# baseline (speedup 1.0000x reference)
"""Trainium2 Bass kernel for nn_MeshTransformer (S=1024, D=512, H=8, L=2).

Sequence-parallel over 8 NeuronCores; v2 restructure:
- Layer 0 K/V recomputed full-S from x0-full (overlaps startup DMA).
- Layer 1: own-shard K/V projections + ONE packed AllGather [1024,128] bf16
  (kT-own c-major + v-own j-major), replacing the x-gather + full K/V
  recompute of the baseline. Q1 overlaps the collective.
- Attention output computed TRANSPOSED: per (h,j) matmul with stationary
  v_nat[j] [128, HD+1] (ones column -> softmax normalizer in row HD) and
  moving exp-scores slice [128,128]; O-projection consumes the per-head
  [64,128] blocks directly (no PE transposes of the attention output).
- FFN2 computed transposed (stationary f2w chunk) -> residual orientation
  directly, no transposes.
- Distance bias via scaled-identity matmuls into the scores PSUM (db1b==0
  collapse), as baseline.
- Evictions spread across Scalar/GpSimd/Vector; 4 chunks per PSUM tile with
  single wide evictions where possible.
- Tiny warmup AllGather first to absorb collective-firmware wakeup/skew.
"""
import numpy as np

S, FEAT, D, H, L, DFF, C = 1024, 64, 512, 8, 2, 2048, 10
DB = D // 4
HD = D // H          # 64 head dim
NCORES = 8
SB = S // NCORES     # 128 own-query block
P = 128
NDCH = D // P        # 4
NFCH = DFF // P      # 16
NJCH = S // P        # 8
VW = HD + 1          # 65: head block width in V (data + ones column)
EPS = 1e-5
USE_RDMA = True   # direct SBUF->SBUF gather instead of CC AllGather

_nc_cache = {}


def _build(flags):
    import concourse.bacc as bacc
    from concourse import mybir, tile

    dt = mybir.dt
    AF = mybir.ActivationFunctionType
    ALU = mybir.AluOpType
    f32 = dt.float32
    b16 = dt.bfloat16
    AX = mybir.AxisListType

    nc = bacc.Bacc("TRN2", num_devices=NCORES, target_bir_lowering=False, debug=False)
    use_rdma = USE_RDMA
    rdma_gate_insts = []
    kv_rsem_holder = []

    def inp(name, shape, dtype=f32):
        return nc.declare_dram_parameter(name, list(shape), dtype, isOutput=False)

    featT_h = inp("featT", [FEAT, S], b16)
    featTo_h = inp("featT_own", [FEAT, SB], b16)
    peT_h = inp("peT", [D, S], b16)
    peTo_h = inp("peT_own", [D, SB])
    Laug_h = inp("Laug", [4, S], b16)
    Laugx_h = inp("Laug_x", [4, S], b16)
    Raug_h = inp("Raug_own", [4, SB], b16)
    sqcol_h = inp("sqcol", [S, 1])
    sqcolx_h = inp("sqcol_x", [S, 1])
    gamT_h = inp("gamT", [P, L * H])
    inw_h = inp("in_w", [FEAT, D], b16)
    inb_h = inp("in_b", [D, 1])
    qw_h = inp("qw2", [L * D, D], b16)
    kw_h = inp("kw2", [L * D, D], b16)
    vw_h = inp("vw2", [L * D, D], b16)
    ow_h = inp("ow2", [L * D, D], b16)
    qb_h = inp("qb2", [L * D, 1])   # pre-scaled by 1/8 on host
    kb_h = inp("kb2", [L * D, 1])
    vb_h = inp("vb2", [L * D, 1])
    ob_h = inp("ob2", [L * D, 1])
    f1w_h = inp("f1w2", [L * D, DFF], b16)
    f2w_h = inp("f2w2", [L * DFF, D], b16)
    f1b_h = inp("f1b2", [L * DFF, 1])
    f2b_h = inp("f2b2", [L * D, 1])
    n1g_h = inp("n1g2", [L * D, 1])
    n1b_h = inp("n1b2", [L * D, 1])
    n2g_h = inp("n2g2", [L * D, 1])
    n2b_h = inp("n2b2", [L * D, 1])
    if not flags["db1b_z"]:
        biasT_h = inp("biasT_own", [L * H * S, SB])

    y_h = nc.declare_dram_parameter("y", [D, 1], f32, isOutput=True)
    import os as _os
    DBG = bool(_os.environ.get("KDBG"))
    dbg_h = {}
    if DBG:
        for nm_ in ["d_xres0", "d_xln0", "d_x2own0", "d_xres1"]:
            dbg_h[nm_] = nc.declare_dram_parameter(nm_, [D, SB], f32, isOutput=True)
        dbg_h["d_at0"] = nc.declare_dram_parameter("d_at0", [D, SB], f32, isOutput=True)
        dbg_h["d_q0"] = nc.declare_dram_parameter("d_q0", [P, D], f32, isOutput=True)
        dbg_h["d_k00"] = nc.declare_dram_parameter("d_k00", [P, S], f32, isOutput=True)
        dbg_h["d_eta0"] = nc.declare_dram_parameter("d_eta0", [P, S], f32, isOutput=True)
        dbg_h["d_v00"] = nc.declare_dram_parameter("d_v00", [P, H * VW], f32, isOutput=True)
        dbg_h["d_ot0"] = nc.declare_dram_parameter("d_ot0", [VW, H * P], f32, isOutput=True)

    with tile.TileContext(nc) as tc:
        with (
            tc.tile_pool(name="const", bufs=1) as cp,
            tc.tile_pool(name="wts", bufs=1) as wp,
            tc.tile_pool(name="act", bufs=1) as ap,
            tc.tile_pool(name="work", bufs=1) as kp,
            tc.tile_pool(name="ps", bufs=1, space="PSUM") as pp,
            tc.tile_pool(name="dram", bufs=1, space="DRAM") as dp,
        ):
            if use_rdma:
                # A collective in the NEFF makes nrt group-launch the 8 cores
                # in sync; without one they start milliseconds apart and the
                # remote-DMA gather waits on the stragglers. Output unused.
                wz = cp.tile([1, 16], f32)
                nc.gpsimd.memset(wz[:], 0.0)
                w_in = dp.tile([1, 16], f32, name="warm_in")
                w_out = dp.tile([NCORES, 16], f32, name="warm_out",
                                addr_space="Shared")
                nc.sync.dma_start(w_in[:], wz[:])
                nc.gpsimd.collective_compute(
                    "AllGather", mybir.AluOpType.bypass,
                    replica_groups=[list(range(NCORES))],
                    ins=[w_in[:].opt()], outs=[w_out[:].opt()])
            # round-robin eviction engines
            def copy_ev(i, dst, src):
                if i % 2 == 0:
                    nc.scalar.activation(dst, src, AF.Copy)
                else:
                    nc.vector.tensor_copy(dst, src)

            def copy_ev2(i, dst, src):
                if i % 2 == 0:
                    return nc.scalar.activation(dst, src, AF.Copy)
                return nc.vector.tensor_copy(dst, src)

            # ---------------- constants / small tiles ----------------
            featT = cp.tile([FEAT, S], b16)
            nc.gpsimd.dma_start(featT[:], featT_h[:, :])
            featTo = cp.tile([FEAT, SB], b16)
            nc.gpsimd.dma_start(featTo[:], featTo_h[:, :])
            inw = cp.tile([FEAT, D], b16)
            nc.gpsimd.dma_start(inw[:], inw_h[:, :])
            Laug = cp.tile([4, S], b16)
            nc.gpsimd.dma_start(Laug[:], Laug_h[:, :])
            Raug = cp.tile([4, SB], b16)
            nc.gpsimd.dma_start(Raug[:], Raug_h[:, :])
            sqc = cp.tile([P, NJCH], f32)
            nc.sync.dma_start(
                sqc[:], sqcol_h[:, :].rearrange("(c p) o -> p (c o)", c=NJCH, p=P))
            if use_rdma:
                Laugx = cp.tile([4, S], b16)
                nc.gpsimd.dma_start(Laugx[:], Laugx_h[:, :])
                sqcx = cp.tile([P, NJCH], f32)
                nc.sync.dma_start(
                    sqcx[:],
                    sqcolx_h[:, :].rearrange("(c p) o -> p (c o)", c=NJCH, p=P))
            gam = cp.tile([P, L * H], f32)
            nc.gpsimd.dma_start(gam[:], gamT_h[:, :])
            peTo = [cp.tile([P, SB], f32, name=f"peTo{d}") for d in range(NDCH)]
            for d in range(NDCH):
                nc.gpsimd.dma_start(peTo[d][:], peTo_h[d * P:(d + 1) * P, :])
            inb = None
            if not flags["in_b_z"]:
                inb = cp.tile([P, NDCH], f32)
                nc.sync.dma_start(
                    inb[:], inb_h[:, :].rearrange("(c p) o -> p (c o)", c=NDCH, p=P))

            # x_full tiles get peT DMA'd in FIRST (before the 12MB of weight
            # DMAs below) so the x0 path isn't starved.
            x_full = [kp.tile([P, S], b16, name=f"xf_{d}") for d in range(NDCH)]
            for d in range(NDCH):
                nc.sync.dma_start(x_full[d][:], peT_h[d * P:(d + 1) * P, :])

            # ---------------- weights (both layers, unique tiles) --------
            # order = DMA priority: layer0 kw/vw first, layer1 f-weights last
            def wload(handle, l, base_rows, nch, width, nm, rows=P):
                ts = []
                for i in range(nch):
                    t = wp.tile([rows, width], b16, name=f"{nm}_{l}_{i}")
                    r0 = l * base_rows + i * rows
                    nc.sync.dma_start(t[:], handle[r0:r0 + rows, :])
                    ts.append(t)
                return ts

            kw0 = wload(kw_h, 0, D, NDCH, D, "kw")
            vw0 = wload(vw_h, 0, D, NDCH, D, "vw")
            qw0 = wload(qw_h, 0, D, NDCH, D, "qw")
            ow0 = wload(ow_h, 0, D, H, D, "ow", rows=HD)
            f1w0 = wload(f1w_h, 0, D, NDCH, DFF, "f1w")
            f2w0 = wload(f2w_h, 0, DFF, NFCH, D, "f2w")
            kw1 = wload(kw_h, 1, D, NDCH, D, "kw")
            vw1 = wload(vw_h, 1, D, NDCH, D, "vw")
            qw1 = wload(qw_h, 1, D, NDCH, D, "qw")
            ow1 = wload(ow_h, 1, D, H, D, "ow", rows=HD)
            f1w1 = wload(f1w_h, 1, D, NDCH, DFF, "f1w")
            f2w1 = wload(f2w_h, 1, DFF, NFCH, D, "f2w")
            QW, KW, VW_, OW = [qw0, qw1], [kw0, kw1], [vw0, vw1], [ow0, ow1]
            F1W, F2W = [f1w0, f1w1], [f2w0, f2w1]

            def lcol(handle, l, nch, name):
                t = cp.tile([P, nch], f32, name=f"{name}{l}")
                nc.sync.dma_start(
                    t[:], handle[l * nch * P:(l + 1) * nch * P, :]
                    .rearrange("(c p) o -> p (c o)", c=nch, p=P))
                return t

            ones_col = cp.tile([P, 1], f32)
            nc.gpsimd.memset(ones_col[:], 1.0)
            ones_colb = cp.tile([P, 1], b16)
            nc.gpsimd.memset(ones_colb[:], 1.0)
            ones_row = cp.tile([1, P], f32)
            nc.gpsimd.memset(ones_row[:], 1.0)
            eps_c = cp.tile([1, 1], f32)
            nc.gpsimd.memset(eps_c[:], EPS)
            identb = cp.tile([P, P], b16)
            nc.gpsimd.memset(identb[:], 1.0)
            nc.gpsimd.affine_select(
                identb[:], identb[:], [[1, P]], ALU.is_equal, 0.0,
                base=0, channel_multiplier=-1)
            # scaled identities gamma[l,h] * I for the distance-bias matmuls
            identg = []
            if flags["db1b_z"]:
                for lh in range(L * H):
                    t = cp.tile([P, P], b16, name=f"identg{lh}")
                    nc.vector.tensor_scalar_mul(t[:], identb[:], gam[:, lh:lh + 1])
                    identg.append(t)

            # V tiles [128, 8*65] persist across layers; ones columns set once.
            v_nat = [kp.tile([P, H * VW], b16, name=f"v_{j}") for j in range(NJCH)]
            for j in range(NJCH):
                nc.gpsimd.memset(v_nat[j][:, HD:H * VW:VW], 1.0)
            # K^T tiles persist across layers (l0: recompute; l1: gathered)
            kT = [kp.tile([P, S], b16, name=f"kT_{d}") for d in range(NDCH)]

            # ---------------- x0 = in-proj + positional enc ----------------
            # (peT already resident in x_full tiles; add in place)
            for d in range(NDCH):
                xt = x_full[d]
                for h2 in range(2):
                    ps = pp.tile([P, 512], f32, name=f"ps_x{d}{h2}", tag="mm", bufs=2)
                    nc.tensor.matmul(
                        ps[:], inw[:, d * P:(d + 1) * P],
                        featT[:, h2 * 512:(h2 + 1) * 512], start=True, stop=True)
                    nc.vector.tensor_add(
                        xt[:, h2 * 512:(h2 + 1) * 512], ps[:],
                        xt[:, h2 * 512:(h2 + 1) * 512])
                if inb is not None:
                    nc.vector.tensor_scalar_add(xt[:], xt[:], inb[:, d:d + 1])

            x_own = []    # 4 tiles [128, 128] f32 - own columns of x (exact spine)
            x_own_b = []  # bf16 copies for matmul rhs
            for d in range(NDCH):
                ps = pp.tile([P, P], f32, name=f"ps_x0o{d}", tag="mm", bufs=2)
                nc.tensor.matmul(ps[:], inw[:, d * P:(d + 1) * P], featTo[:],
                                 start=True, stop=True)
                xo = kp.tile([P, SB], f32, name=f"xo0_{d}", tag=f"xo{d}")
                nc.vector.tensor_add(xo[:], ps[:], peTo[d][:])
                if inb is not None:
                    nc.vector.tensor_scalar_add(xo[:], xo[:], inb[:, d:d + 1])
                x_own.append(xo)
                xb = kp.tile([P, SB], b16, name=f"xo0b_{d}", tag=f"xob{d}")
                nc.vector.tensor_copy(xb[:], xo[:])
                x_own_b.append(xb)

            # ---------------- pairwise distances (own block, bf16) ----------
            def build_dist(Lsrc, sqsrc, nm):
                ts = []
                for j in range(NJCH):
                    ps = pp.tile([P, P], f32, name=f"ps_d{nm}{j}", tag="mm", bufs=2)
                    nc.tensor.matmul(ps[:], Lsrc[:, j * P:(j + 1) * P], Raug[:],
                                     start=True, stop=True)
                    dsq = ap.tile([P, SB], f32, name=f"dsq{nm}{j}",
                                  tag="dsq", bufs=2)
                    nc.vector.tensor_scalar(
                        dsq[:], ps[:], sqsrc[:, j:j + 1], 0.0, ALU.add, ALU.max)
                    dtl = kp.tile([P, SB], b16, name=f"distT{nm}{j}",
                                  tag=f"distT{j}")
                    nc.scalar.activation(dtl[:], dsq[:], AF.Sqrt)
                    ts.append(dtl)
                return ts

            distT = build_dist(Laug, sqc, "g")

            # layer-1 K/V gather buffers
            if use_rdma:
                # slot s holds the pack of core (self XOR s); slot 0 == self,
                # written locally by the K/V-own evictions. Per-slot arrival
                # semaphores so consumption can start as slots arrive.
                kv_arena = kp.tile([P, NCORES * 2 * D], b16, name="kv_arena")
                kv_rsems = [None] + [nc.alloc_semaphore(f"kv_rsem{s}")
                                     for s in range(1, NCORES)]
                kv_lsem = nc.alloc_semaphore("kv_lsem")
                kv_rsem_holder.append(kv_rsems)
            else:
                kv_dram = dp.tile([P, 2 * D], b16, name="kv_dram")
                kv_gath = dp.tile([NCORES * P, 2 * D], b16,
                                  name="kv_gath", addr_space="Shared")

            def layernorm(xin, g, b, nm, want_f32=True, stats_only=False):
                """f32 stats via matmul; returns (f32 tiles or None, bf16 tiles)."""
                sqs = []
                for d in range(NDCH):
                    sq = ap.tile([P, SB], b16, name=f"sq{nm}{d}",
                                 tag="lnsq", bufs=2)
                    nc.vector.tensor_mul(sq[:], xin[d][:], xin[d][:])
                    sqs.append(sq)
                sr = pp.tile([1, P], f32, name=f"ps_sr{nm}", tag="mm", bufs=2)
                for d in range(NDCH):
                    nc.tensor.matmul(sr[:], ones_col[:], xin[d][:],
                                     start=(d == 0), stop=(d == NDCH - 1))
                s2 = pp.tile([1, P], f32, name=f"ps_s2{nm}", tag="mm", bufs=2)
                for d in range(NDCH):
                    nc.tensor.matmul(s2[:], ones_colb[:], sqs[d][:],
                                     start=(d == 0), stop=(d == NDCH - 1))
                mu = ap.tile([1, P], f32, name=f"mu{nm}", tag="lnrow", bufs=4)
                nc.vector.tensor_scalar_mul(mu[:], sr[:], 1.0 / D)
                em = ap.tile([1, P], f32, name=f"em{nm}", tag="lnrow", bufs=4)
                nc.vector.tensor_scalar_mul(em[:], s2[:], 1.0 / D)
                mu2 = ap.tile([1, P], f32, name=f"mu2{nm}", tag="lnrow", bufs=4)
                nc.vector.tensor_mul(mu2[:], mu[:], mu[:])
                var = ap.tile([1, P], f32, name=f"var{nm}", tag="lnrow", bufs=4)
                nc.vector.tensor_sub(var[:], em[:], mu2[:])
                sd = ap.tile([1, P], f32, name=f"sd{nm}", tag="lnrow", bufs=4)
                nc.scalar.activation(sd[:], var[:], AF.Sqrt, bias=eps_c[:])
                rstd = ap.tile([1, P], f32, name=f"rstd{nm}", tag="lnrow", bufs=4)
                nc.vector.reciprocal(rstd[:], sd[:])
                if stats_only:
                    return None, None, mu, rstd
                mub_p = pp.tile([P, P], f32, name=f"ps_mub{nm}", tag="mm", bufs=2)
                nc.tensor.matmul(mub_p[:], ones_row[:], mu[:], start=True, stop=True)
                rsb_p = pp.tile([P, P], f32, name=f"ps_rsb{nm}", tag="mm", bufs=2)
                nc.tensor.matmul(rsb_p[:], ones_row[:], rstd[:], start=True, stop=True)
                mub = ap.tile([P, P], f32, name=f"mub{nm}", tag="lnmub", bufs=2)
                nc.scalar.activation(mub[:], mub_p[:], AF.Copy)
                rsb = ap.tile([P, P], f32, name=f"rsb{nm}", tag="lnrsb", bufs=2)
                nc.vector.tensor_copy(rsb[:], rsb_p[:])
                outs, outsb = [], []
                for d in range(NDCH):
                    t16 = ap.tile([P, SB], f32, name=f"lnt{nm}{d}",
                                  tag="lntmp", bufs=2)
                    nc.vector.tensor_sub(t16[:], xin[d][:], mub[:])
                    ob_ = kp.tile([P, SB], b16, name=f"lnb{nm}{d}",
                                  tag=f"lnb{nm[0]}{d}")
                    nc.vector.tensor_mul(ob_[:], t16[:], rsb[:])
                    if g is not None or b is not None:
                        gcol = g[:, d:d + 1] if g is not None else 1.0
                        bcol = b[:, d:d + 1] if b is not None else 0.0
                        nc.vector.tensor_scalar(
                            ob_[:], ob_[:], gcol, bcol, ALU.mult, ALU.add)
                    outsb.append(ob_)
                    if want_f32:
                        o = kp.tile([P, SB], f32, name=f"ln{nm}{d}",
                                    tag=f"ln{nm[0]}{d}")
                        nc.vector.tensor_mul(o[:], t16[:], rsb[:])
                        if g is not None or b is not None:
                            gcol = g[:, d:d + 1] if g is not None else 1.0
                            bcol = b[:, d:d + 1] if b is not None else 0.0
                            nc.vector.tensor_scalar(
                                o[:], o[:], gcol, bcol, ALU.mult, ALU.add)
                        outs.append(o)
                return outs, outsb, mu, rstd

            # ---------------- layers ----------------
            for l in range(L):
                qw, kw, vw, ow = QW[l], KW[l], VW_[l], OW[l]
                f1w, f2w = F1W[l], F2W[l]
                qb = None if flags["qb_z"] else lcol(qb_h, l, NDCH, "qb")
                kb = None if flags["kb_z"] else lcol(kb_h, l, NDCH, "kb")
                ob = None if flags["ob_z"] else lcol(ob_h, l, NDCH, "ob")
                f1b = None if flags["f1b_z"] else lcol(f1b_h, l, NFCH, "f1b")
                f2b = None if flags["f2b_z"] else lcol(f2b_h, l, NDCH, "f2b")
                n1g = None if flags["n1g_1"] else lcol(n1g_h, l, NDCH, "n1g")
                n1b = None if flags["n1b_z"] else lcol(n1b_h, l, NDCH, "n1b")
                n2g = None if flags["n2g_1"] else lcol(n2g_h, l, NDCH, "n2g")
                n2b = None if flags["n2b_z"] else lcol(n2b_h, l, NDCH, "n2b")
                vbr = None
                if not flags["vb_z"]:
                    vbr = cp.tile([1, D], f32, name=f"vbr{l}")
                    nc.sync.dma_start(
                        vbr[:], vb_h[l * D:(l + 1) * D, :].rearrange("p o -> o p"))

                if l == 0:
                    # -- K^T full recompute (layer 0 only) --
                    ei = 0
                    for d in range(NDCH):
                        for h2 in range(2):
                            ps = pp.tile([P, 512], f32, name=f"ps_k{l}{d}{h2}",
                                         tag="mm", bufs=2)
                            for dk in range(NDCH):
                                nc.tensor.matmul(
                                    ps[:], kw[dk][:, d * P:(d + 1) * P],
                                    x_full[dk][:, h2 * 512:(h2 + 1) * 512],
                                    start=(dk == 0), stop=(dk == NDCH - 1))
                            if kb is not None:
                                nc.scalar.activation(
                                    kT[d][:, h2 * 512:(h2 + 1) * 512], ps[:],
                                    AF.Copy, bias=kb[:, d:d + 1])
                            else:
                                copy_ev(ei, kT[d][:, h2 * 512:(h2 + 1) * 512], ps[:])
                            ei += 1
                    # -- V full recompute (layer 0 only) --
                    for j in range(NJCH):
                        ps = pp.tile([P, D], f32, name=f"ps_v{l}{j}", tag="mm", bufs=2)
                        for dk in range(NDCH):
                            nc.tensor.matmul(
                                ps[:], x_full[dk][:, j * P:(j + 1) * P], vw[dk][:],
                                start=(dk == 0), stop=(dk == NDCH - 1 and vbr is None))
                        if vbr is not None:
                            nc.tensor.matmul(ps[:], ones_row[:], vbr[:],
                                             start=False, stop=True)
                        copy_ev(
                            j,
                            v_nat[j][:, :].rearrange(
                                "p (h c) -> p h c", c=VW)[:, :, 0:HD],
                            ps[:, :].rearrange("p (h c) -> p h c", c=HD))
                else:
                    if use_rdma:
                        # layer-1 j-slots are XOR-ordered; rebuild distances
                        # (reuses the distT tile buffers; l0 reads are done)
                        distT = build_dist(Laugx, sqcx, "x")
                    # -- layer 1: own-shard K/V + packed AllGather --
                    # kT-own: [128, 512] regions d = [c-rows of chunk d, own j]
                    kps = pp.tile([P, D], f32, name="ps_ko", tag="mm", bufs=2)
                    for d in range(NDCH):
                        for dk in range(NDCH):
                            nc.tensor.matmul(
                                kps[:, d * P:(d + 1) * P],
                                kw[dk][:, d * P:(d + 1) * P], x_own_b[dk][:],
                                start=(dk == 0), stop=(dk == NDCH - 1))
                    if use_rdma:
                        kto = kv_arena[:, 0:D]
                    else:
                        kto_t = ap.tile([P, D], b16, name="kto", tag="kto")
                        kto = kto_t[:]
                    if kb is not None:
                        for d in range(NDCH):
                            nc.scalar.activation(
                                kto[:, d * P:(d + 1) * P],
                                kps[:, d * P:(d + 1) * P], AF.Copy,
                                bias=kb[:, d:d + 1])
                    else:
                        nc.scalar.activation(kto, kps[:], AF.Copy)
                    if not use_rdma:
                        nc.sync.dma_start(kv_dram[:, 0:D], kto)
                    # v-own natural [own j, c]
                    vps = pp.tile([P, D], f32, name="ps_vo", tag="mm", bufs=2)
                    for dk in range(NDCH):
                        nc.tensor.matmul(
                            vps[:], x_own_b[dk][:], vw[dk][:],
                            start=(dk == 0), stop=(dk == NDCH - 1 and vbr is None))
                    if vbr is not None:
                        nc.tensor.matmul(vps[:], ones_row[:], vbr[:],
                                         start=False, stop=True)
                    if use_rdma:
                        vno = kv_arena[:, D:2 * D]
                    else:
                        vno_t = ap.tile([P, D], b16, name="vno", tag="vno")
                        vno = vno_t[:]
                    nc.vector.tensor_copy(vno, vps[:])
                    if use_rdma:
                        # 8 single-dest broadcasts: slot s on every receiver
                        # gets the pack of core (self XOR s). Inputs
                        # (featT/peT/Laug/sqcol) are XOR-permuted per core so
                        # slot indices are consistent across layers.
                        for s in range(1, NCORES):
                            rd = [None] * NCORES
                            rd[s] = (0, s)
                            nc.gpsimd.remote_dma_broadcast(
                                kv_arena[:, s * 2 * D:(s + 1) * 2 * D],
                                kv_arena[:, 0:2 * D],
                                kv_rsems[s], kv_lsem, rdests=rd)
                        tc.no_sync_barrier()
                        nc.gpsimd.trigger_dma(count=None)
                    else:
                        nc.sync.dma_start(kv_dram[:, D:2 * D], vno)
                        nc.gpsimd.collective_compute(
                            "AllGather", mybir.AluOpType.bypass,
                            replica_groups=[list(range(NCORES))],
                            ins=[kv_dram[:].opt()], outs=[kv_gath[:].opt()])

                # -- Q^T own (pre-scaled by 1/8): [128, 512] regions --
                qps = pp.tile([P, D], f32, name=f"ps_q{l}", tag="mm", bufs=2)
                for d in range(NDCH):
                    for dk in range(NDCH):
                        nc.tensor.matmul(
                            qps[:, d * P:(d + 1) * P],
                            qw[dk][:, d * P:(d + 1) * P], x_own_b[dk][:],
                            start=(dk == 0), stop=(dk == NDCH - 1))
                qT = ap.tile([P, D], b16, name=f"qT{l}", tag="qT")
                if qb is not None:
                    for d in range(NDCH):
                        nc.scalar.activation(
                            qT[:, d * P:(d + 1) * P], qps[:, d * P:(d + 1) * P],
                            AF.Copy, scale=0.125, bias=qb[:, d:d + 1])
                else:
                    nc.scalar.activation(qT[:], qps[:], AF.Copy, scale=0.125)

                if l == 1:
                    # assemble gathered K/V: per peer-slot r, K cols + V block
                    for r in range(NCORES):
                        if use_rdma:
                            base = r * 2 * D
                            for d in range(NDCH):
                                inst = (nc.vector.tensor_copy
                                        if d % 2 else nc.scalar.copy)(
                                    kT[d][:, r * P:(r + 1) * P],
                                    kv_arena[:, base + d * P:base + (d + 1) * P])
                                if r:
                                    inst.wait_op(kv_rsems[r], 0, "sem-ge")
                                    rdma_gate_insts.append((inst, r))
                            inst = copy_ev2(
                                r,
                                v_nat[r][:, :].rearrange(
                                    "p (h c) -> p h c", c=VW)[:, :, 0:HD],
                                kv_arena[:, base + D:base + 2 * D].rearrange(
                                    "p (h c) -> p h c", c=HD))
                            if r:
                                inst.wait_op(kv_rsems[r], 0, "sem-ge")
                                rdma_gate_insts.append((inst, r))
                        else:
                            r0 = r * P
                            for d in range(NDCH):
                                nc.sync.dma_start(
                                    kT[d][:, r * P:(r + 1) * P],
                                    kv_gath[r0:r0 + P, d * P:(d + 1) * P])
                            vb_t = ap.tile([P, D], b16, name=f"vb{r}",
                                           tag="vb", bufs=3)
                            nc.sync.dma_start(vb_t[:],
                                              kv_gath[r0:r0 + P, D:2 * D])
                            copy_ev(
                                r,
                                v_nat[r][:, :].rearrange(
                                    "p (h c) -> p h c", c=VW)[:, :, 0:HD],
                                vb_t[:, :].rearrange("p (h c) -> p h c", c=HD))

                # -- attention: scores+bias in PSUM, exp, transposed e@[V|1] --
                oT = pp.tile([VW, H * P], f32, name=f"ps_oT{l}", tag="oT", bufs=1)
                eTas = []
                for j in range(NJCH):
                    scA = pp.tile([P, S], f32, name=f"ps_scA{l}{j}",
                                  tag="scA", bufs=2)
                    for h in range(H):
                        t2, off = h // 2, HD * (h % 2)
                        nc.tensor.matmul(
                            scA[:, h * P:(h + 1) * P],
                            kT[t2][off:off + HD, j * P:(j + 1) * P],
                            qT[off:off + HD, t2 * P:(t2 + 1) * P],
                            start=True, stop=not flags["db1b_z"])
                        if flags["db1b_z"]:
                            nc.tensor.matmul(
                                scA[:, h * P:(h + 1) * P],
                                identg[l * H + h][:], distT[j][:],
                                start=False, stop=True)
                    if not flags["db1b_z"]:
                        bt = ap.tile([P, S], f32, name=f"bt{l}{j}", tag="bt", bufs=2)
                        for h in range(H):
                            r0 = ((l * H + h) * NJCH + j) * P
                            nc.sync.dma_start(
                                bt[:, h * P:(h + 1) * P], biasT_h[r0:r0 + P, :])
                        lg = ap.tile([P, S], f32, name=f"lg{l}{j}", tag="lg", bufs=2)
                        nc.vector.tensor_add(lg[:], scA[:], bt[:])
                        src = lg
                    else:
                        src = scA
                    if j < 4:
                        eTa = kp.tile([P, S], b16, name=f"eTa{l}{j}",
                                      tag=f"xf_{j}")
                    else:
                        eTa = ap.tile([P, S], b16, name=f"eTa{l}{j}",
                                      tag=f"eTa{j}")
                    nc.scalar.activation(eTa[:], src[:], AF.Exp)
                    eTas.append(eTa)
                    # j-outer accumulation: slot j's oT contribution lands
                    # right after its exp, so layer-1 attention pipelines
                    # behind the staggered K/V arrivals.
                    for h in range(H):
                        nc.tensor.matmul(
                            oT[:, h * P:(h + 1) * P],
                            v_nat[j][:, h * VW:(h + 1) * VW],
                            eTa[:, h * P:(h + 1) * P],
                            start=(j == 0), stop=(j == NJCH - 1))

                # normalizers: row HD of each head block
                rvh = []
                for h in range(H):
                    rv = ap.tile([1, P], f32, name=f"rv{l}{h}", tag=f"rv{h}")
                    nc.vector.reciprocal(
                        rv[:], oT[HD:HD + 1, h * P:(h + 1) * P])
                    rvh.append(rv)
                # R_h = broadcast of rv_h to 64 partitions (via matmul)
                r8 = ap.tile([HD, H * P], f32, name=f"r8{l}", tag="r8")
                for g in range(2):
                    rps = pp.tile([HD, 512], f32, name=f"ps_R{l}{g}",
                                  tag="mm", bufs=2)
                    for hh in range(4):
                        h = 4 * g + hh
                        nc.tensor.matmul(rps[:, hh * P:(hh + 1) * P],
                                         ones_row[:, 0:HD], rvh[h][:],
                                         start=True, stop=True)
                    nc.vector.tensor_copy(
                        r8[:, g * 512:(g + 1) * 512], rps[:])
                attnT = []   # 8 tiles [64, 128] bf16, normalized head outputs
                for h in range(H):
                    at = ap.tile([HD, P], b16, name=f"at{l}{h}", tag=f"at{h}")
                    nc.vector.tensor_mul(
                        at[:], oT[0:HD, h * P:(h + 1) * P],
                        r8[:, h * P:(h + 1) * P])
                    attnT.append(at)

                if DBG and l == 0:
                    for h in range(H):
                        nc.gpsimd.dma_start(
                            dbg_h["d_at0"][h * HD:(h + 1) * HD, :], attnT[h][:])
                    nc.gpsimd.dma_start(dbg_h["d_q0"][:, :], qT[:])
                    nc.gpsimd.dma_start(dbg_h["d_k00"][:, :], kT[0][:])
                    nc.gpsimd.dma_start(dbg_h["d_eta0"][:, :], eTas[0][:])
                    nc.gpsimd.dma_start(dbg_h["d_v00"][:, :], v_nat[0][:])
                    ot_s = ap.tile([VW, H * P], f32, name="ot_s")
                    nc.vector.tensor_copy(ot_s[:], oT[:])
                    nc.sync.dma_start(dbg_h["d_ot0"][:, :], ot_s[:])

                # -- O-projection (per-head chunks) + residual --
                ops = pp.tile([P, D], f32, name=f"ps_o{l}", tag="mm", bufs=2)
                for d in range(NDCH):
                    for h in range(H):
                        nc.tensor.matmul(
                            ops[:, d * P:(d + 1) * P],
                            ow[h][:, d * P:(d + 1) * P],
                            attnT[h][:],
                            start=(h == 0), stop=(h == H - 1))
                xres = []
                for d in range(NDCH):
                    xr = kp.tile([P, SB], f32, name=f"xr1_{l}_{d}", tag=f"xr1{d}")
                    nc.vector.tensor_add(xr[:], ops[:, d * P:(d + 1) * P],
                                         x_own[d][:])
                    if ob is not None:
                        nc.vector.tensor_scalar_add(xr[:], xr[:], ob[:, d:d + 1])
                    xres.append(xr)
                if DBG and l <= 1:
                    for d in range(NDCH):
                        nc.sync.dma_start(
                            dbg_h[f"d_xres{l}"][d * P:(d + 1) * P, :], xres[d][:])

                x_ln, x_ln_b, _, _ = layernorm(xres, n1g, n1b, f"a{l}")
                if DBG and l == 0:
                    for d in range(NDCH):
                        nc.sync.dma_start(
                            dbg_h["d_xln0"][d * P:(d + 1) * P, :], x_ln[d][:])

                # -- FFN1: 4 f-chunks per PSUM tile, single wide relu-evict --
                h1q = []
                for t in range(NFCH // 4):
                    ps = pp.tile([P, 512], f32, name=f"ps_f1{l}{t}", tag="mm", bufs=2)
                    for q in range(4):
                        f = 4 * t + q
                        for d in range(NDCH):
                            nc.tensor.matmul(
                                ps[:, q * P:(q + 1) * P],
                                f1w[d][:, f * P:(f + 1) * P], x_ln_b[d][:],
                                start=(d == 0), stop=(d == NDCH - 1))
                    hq = ap.tile([P, 512], b16, name=f"h1_{l}_{t}", tag=f"h1{t}")
                    if f1b is not None:
                        for q in range(4):
                            f = 4 * t + q
                            nc.scalar.activation(
                                hq[:, q * P:(q + 1) * P], ps[:, q * P:(q + 1) * P],
                                AF.Relu, bias=f1b[:, f:f + 1])
                    else:
                        if t % 2:
                            nc.vector.tensor_scalar_max(hq[:], ps[:], 0.0)
                        else:
                            nc.scalar.activation(hq[:], ps[:], AF.Relu)
                    h1q.append(hq)
                # -- FFN2 transposed: output [d, i] directly --
                f2ps = pp.tile([P, D], f32, name=f"ps_f2{l}", tag="mm", bufs=2)
                for d in range(NDCH):
                    for f in range(NFCH):
                        nc.tensor.matmul(
                            f2ps[:, d * P:(d + 1) * P],
                            f2w[f][:, d * P:(d + 1) * P],
                            h1q[f // 4][:, (f % 4) * P:(f % 4 + 1) * P],
                            start=(f == 0), stop=(f == NFCH - 1))
                xres2 = []
                for d in range(NDCH):
                    xr = kp.tile([P, SB], f32, name=f"xr2_{l}_{d}", tag=f"xr2{d}")
                    nc.vector.tensor_add(xr[:], f2ps[:, d * P:(d + 1) * P],
                                         x_ln[d][:])
                    if f2b is not None:
                        nc.vector.tensor_scalar_add(xr[:], xr[:], f2b[:, d:d + 1])
                    xres2.append(xr)

                if l + 1 < L:
                    x_own, x_own_b, _, _ = layernorm(xres2, n2g, n2b, f"b{l}")
                    if DBG:
                        for d in range(NDCH):
                            nc.sync.dma_start(
                                dbg_h["d_x2own0"][d * P:(d + 1) * P, :],
                                x_own[d][:])
                else:
                    if False and flags["n2g_1"] and flags["n2b_z"]:
                        # final LN: only the pooled row-sum of the output is
                        # needed -> accumulate (x - mu)*rstd directly.
                        _, _, mu, rstd = layernorm(
                            xres2, None, None, f"b{l}", stats_only=True)
                        mr = ap.tile([1, P], f32, name="mr", tag="lnrow", bufs=4)
                        nc.vector.tensor_mul(mr[:], mu[:], rstd[:])
                        smr = ap.tile([1, 1], f32, name="smr")
                        nc.vector.tensor_reduce(
                            smr[:], mr[:], AX.X, ALU.add)
                        smrb = pp.tile([P, 1], f32, name="ps_smrb", tag="mm", bufs=2)
                        nc.tensor.matmul(smrb[:], ones_row[:], smr[:],
                                         start=True, stop=True)
                        rsb2 = pp.tile([P, P], f32, name="ps_rsb2", tag="mm", bufs=2)
                        nc.tensor.matmul(rsb2[:], ones_row[:], rstd[:],
                                         start=True, stop=True)
                        for d in range(NDCH):
                            scr = ap.tile([P, SB], f32, name=f"scr{d}",
                                          tag="scr", bufs=2)
                            acc = ap.tile([P, 1], f32, name=f"acc{d}",
                                          tag="acc", bufs=4)
                            nc.vector.scalar_tensor_tensor(
                                scr[:], xres2[d][:], 1.0, rsb2[:],
                                ALU.mult, ALU.mult, accum_out=acc[:])
                            red = ap.tile([P, 1], f32, name=f"red{d}",
                                          tag="red", bufs=4)
                            nc.vector.tensor_sub(red[:], acc[:], smrb[:])
                            nc.sync.dma_start(y_h[d * P:(d + 1) * P, :], red[:])
                    else:
                        x_fin, _, _, _ = layernorm(xres2, n2g, n2b, f"b{l}")
                        for d in range(NDCH):
                            red = ap.tile([P, 1], f32, name=f"red{d}",
                                          tag="red", bufs=4)
                            nc.vector.reduce_sum(red[:], x_fin[d][:], axis=AX.X)
                            nc.sync.dma_start(y_h[d * P:(d + 1) * P, :], red[:])

    # Arm the remote-data-arrival gates AFTER tile scheduling: the waits were
    # built with value 0 (trivially satisfiable -- the scheduler's single-core
    # sim cannot model cross-core sem updates and would deadlock on the real
    # value). Each slot's transfer bumps its own rsem by 2, so slot r's
    # consumers arm at 2 and can start as soon as THAT slot arrived.
    if use_rdma:
        kv_rsems_f = kv_rsem_holder[0]
        armed = 0
        for inst, r in rdma_gate_insts:
            si = inst.ins.sync_info
            assert si is not None
            for w in si.on_wait:
                if w.ant_name == kv_rsems_f[r].name:
                    w.wait_value = 2
                    armed += 1
        assert armed == len(rdma_gate_insts), (armed, len(rdma_gate_insts))
    nc.compile()
    return nc


def _prep(inputs):
    """Host-side input prep: transposes, positional encoding, bias collapse."""
    import ml_dtypes
    f32 = np.float32
    bf16 = ml_dtypes.bfloat16
    pos = np.asarray(inputs["positions"], f32)          # [S, 3]
    feat = np.asarray(inputs["features"], f32)          # [S, FEAT]
    fb = np.asarray(inputs["freq_bands"], f32)          # [NFREQ]

    enc = []
    for i in range(3):
        cs = pos[:, i:i + 1] * fb[None, :]
        enc.append(np.sin(cs, dtype=f32))
        enc.append(np.cos(cs, dtype=f32))
    pe = np.concatenate(enc, axis=-1).astype(f32)
    if pe.shape[1] < D:
        pe = np.pad(pe, ((0, 0), (0, D - pe.shape[1])))
    peT = np.ascontiguousarray(pe.T)                    # [D, S]

    featT = np.ascontiguousarray(feat.T)                # [FEAT, S]
    posT = np.ascontiguousarray(pos.T)                  # [3, S]
    sq = (pos * pos).sum(1).astype(f32)                 # [S]
    Laug = np.concatenate([-2.0 * posT, np.ones((1, S), f32)], 0)
    Raug = np.concatenate([posT, sq[None, :]], 0)

    db1w = np.asarray(inputs["db1w"], f32)
    db1b = np.asarray(inputs["db1b"], f32)
    db2w = np.asarray(inputs["db2w"], f32)
    db1b_z = bool(np.all(db1b == 0))
    gam = np.zeros((L, H), f32)
    biasT_own = None
    if db1b_z:
        for l in range(L):
            gam[l] = np.maximum(db1w[l, 0], 0.0) @ db2w[l]
    else:
        diff = pos[:, None, :] - pos[None, :, :]
        sqm = np.sum(diff * diff, axis=-1)
        dist = np.sqrt(np.where(sqm > 0, sqm, 1.0)).astype(f32) * (sqm > 0)
        biasT_own = np.zeros((NCORES, L * H * S, SB), f32)
        for l in range(L):
            hbl = np.maximum(dist[:, :, None] * db1w[l, 0][None, None, :]
                             + db1b[l][None, None, :], 0.0).astype(f32)
            bl = np.einsum("ijc,ch->hij", hbl, db2w[l]).astype(f32)
            for c in range(NCORES):
                blk = bl[:, c * SB:(c + 1) * SB, :]
                biasT_own[c, l * H * S:(l + 1) * H * S, :] = (
                    blk.transpose(0, 2, 1).reshape(H * S, SB))
    gamT = np.broadcast_to(gam.reshape(1, L * H), (P, L * H)).copy()

    def col(x):
        return np.ascontiguousarray(np.asarray(x, f32).reshape(-1, 1))

    common = {
        "featT": featT.astype(bf16),
        "peT": peT.astype(bf16),
        "Laug": Laug.astype(bf16),
        "Laug_x": Laug.astype(bf16),
        "sqcol": col(sq),
        "sqcol_x": col(sq),
        "gamT": gamT,
        "in_w": np.asarray(inputs["in_w"], f32).astype(bf16),
        "in_b": col(inputs["in_b"]),
        "qw2": np.asarray(inputs["qw"], f32).reshape(L * D, D).astype(bf16),
        "kw2": np.asarray(inputs["kw"], f32).reshape(L * D, D).astype(bf16),
        "vw2": np.asarray(inputs["vw"], f32).reshape(L * D, D).astype(bf16),
        "ow2": np.asarray(inputs["ow"], f32).reshape(L * D, D).astype(bf16),
        "qb2": col(np.asarray(inputs["qb"], f32) * 0.125),
        "kb2": col(inputs["kb"]),
        "vb2": col(inputs["vb"]),
        "ob2": col(inputs["ob"]),
        "f1w2": np.asarray(inputs["f1w"], f32).reshape(L * D, DFF).astype(bf16),
        "f2w2": np.asarray(inputs["f2w"], f32).reshape(L * DFF, D).astype(bf16),
        "f1b2": col(inputs["f1b"]),
        "f2b2": col(inputs["f2b"]),
        "n1g2": col(inputs["n1g"]),
        "n1b2": col(inputs["n1b"]),
        "n2g2": col(inputs["n2g"]),
        "n2b2": col(inputs["n2b"]),
    }
    flags = {
        "in_b_z": bool(np.all(common["in_b"] == 0)),
        "qb_z": bool(np.all(common["qb2"] == 0)),
        "kb_z": bool(np.all(common["kb2"] == 0)),
        "vb_z": bool(np.all(common["vb2"] == 0)),
        "ob_z": bool(np.all(common["ob2"] == 0)),
        "f1b_z": bool(np.all(common["f1b2"] == 0)),
        "f2b_z": bool(np.all(common["f2b2"] == 0)),
        "n1g_1": bool(np.all(common["n1g2"] == 1)),
        "n1b_z": bool(np.all(common["n1b2"] == 0)),
        "n2g_1": bool(np.all(common["n2g2"] == 1)),
        "n2b_z": bool(np.all(common["n2b2"] == 0)),
        "db1b_z": db1b_z,
    }
    in_maps = []
    for c in range(NCORES):
        m = dict(common)
        m["featT_own"] = np.ascontiguousarray(
            featT[:, c * SB:(c + 1) * SB]).astype(bf16)
        m["peT_own"] = np.ascontiguousarray(peT[:, c * SB:(c + 1) * SB])
        m["Raug_own"] = np.ascontiguousarray(
            Raug[:, c * SB:(c + 1) * SB]).astype(bf16)
        if USE_RDMA:
            # XOR-slot order for layer 1: slot s holds global block (c ^ s)
            perm = np.concatenate(
                [np.arange((c ^ s) * SB, (c ^ s) * SB + SB)
                 for s in range(NCORES)])
            m["Laug_x"] = np.ascontiguousarray(Laug[:, perm]).astype(bf16)
            m["sqcol_x"] = col(sq[perm])
            if biasT_own is not None:
                bt = biasT_own[c].reshape(L, H, NJCH, SB, SB)
                border = [c ^ s for s in range(NCORES)]
                bt1 = bt.copy()
                bt1[1] = bt[1][:, border]
                m["biasT_own"] = np.ascontiguousarray(
                    bt1.reshape(L * H * S, SB))
        if biasT_own is not None and "biasT_own" not in m:
            m["biasT_own"] = biasT_own[c]
        in_maps.append(m)
    return flags, in_maps


def get_nc_and_inmaps(inputs):
    flags, in_maps = _prep(inputs)
    key = tuple(sorted(flags.items()))
    if key not in _nc_cache:
        _nc_cache[key] = _build(flags)
    return _nc_cache[key], in_maps


def finish_output(res, inputs):
    f32 = np.float32
    pooled = np.zeros((D,), f32)
    for c in range(NCORES):
        pooled += np.asarray(res.results[c]["y"], f32).reshape(D)
    pooled /= S
    z = np.maximum(pooled @ np.asarray(inputs["c1w"], f32)
                   + np.asarray(inputs["c1b"], f32), 0.0)
    y = z @ np.asarray(inputs["c2w"], f32) + np.asarray(inputs["c2b"], f32)
    return y.reshape(1, C).astype(f32)


def kernel(**inputs) -> np.ndarray:
    from concourse import bass_utils
    nc, in_maps = get_nc_and_inmaps(inputs)
    res = bass_utils.run_bass_kernel_spmd(
        nc, in_maps, core_ids=list(range(NCORES)))
    return finish_output(res, inputs)


if __name__ == "__main__":
    import jax
    cpu = jax.devices("cpu")[0]
    with jax.default_device(cpu):
        import reference
        inputs = {k: np.asarray(jax.device_put(np.asarray(v), cpu))
                  for k, v in reference.setup_inputs().items()}
        exp = np.asarray(reference.reference(**inputs))
    out = kernel(**inputs)
    err = np.abs(out - exp).max() / (np.abs(exp).max() + 1e-12)
    print("out:", out)
    print("exp:", exp)
    print("rel err:", err)

